# revision 1
# baseline (speedup 1.0000x reference)
"""MHA Bass kernel for TRN2, 8 NeuronCores.

Sharding: data-parallel on batch (2) x tensor-parallel on heads (4 groups of 4
heads). Core c handles batch c//4 and heads 4*(c%4)..4*(c%4)+3 (columns
m0=256*(c%4)). LayerNorm is folded into the projections as a rank-1
correction (gamma folded into weights on host; mean/var computed on-device
via ones-matmuls); attention computed with transposed scores (S^T) so the
softmax'd P^T feeds the O-matmul directly; softmax denominators ride the
O-matmul as a ones-row (M=65); out-projection partials are ReduceScattered
across each batch group of 4 cores.
"""
import numpy as np

B, LQ, D = 2, 2048, 1024
NHEAD, DHEAD = 16, 64
NC = 8
GPC = 4              # cores per batch group
MPC = 256            # output cols per core
N_DCH = D // 128     # 8 d-chunks
N_TCH = LQ // 128    # 16 token chunks
N_TT = LQ // 512     # 4 token tiles of 512
HPC = 4              # heads per core

_NC_CACHE = [None]


def _build():
    import concourse.bacc as bacc
    import concourse.mybir as mybir
    from concourse import tile

    f32, bf16 = mybir.dt.float32, mybir.dt.bfloat16
    AF = mybir.ActivationFunctionType
    MUL, ADD, SUB = mybir.AluOpType.mult, mybir.AluOpType.add, mybir.AluOpType.subtract

    nc = bacc.Bacc("TRN2", target_bir_lowering=False, debug=False, num_devices=NC)

    xq = nc.dram_tensor("xq", [LQ, D], f32, kind="ExternalInput").ap()
    xk = nc.dram_tensor("xk", [LQ, D], f32, kind="ExternalInput").ap()
    xv = nc.dram_tensor("xv", [LQ, D], f32, kind="ExternalInput").ap()
    wqT = nc.dram_tensor("wqT", [D, MPC], bf16, kind="ExternalInput").ap()
    wkT = nc.dram_tensor("wkT", [D, MPC], bf16, kind="ExternalInput").ap()
    wvT = nc.dram_tensor("wvT", [D, MPC], bf16, kind="ExternalInput").ap()
    wgT = nc.dram_tensor("wgT", [D, MPC], bf16, kind="ExternalInput").ap()
    woT = nc.dram_tensor("woT", [MPC, D], bf16, kind="ExternalInput").ap()
    mucq = nc.dram_tensor("mucq", [1, MPC], bf16, kind="ExternalInput").ap()
    muck = nc.dram_tensor("muck", [1, MPC], bf16, kind="ExternalInput").ap()
    mucv = nc.dram_tensor("mucv", [1, MPC], bf16, kind="ExternalInput").ap()
    mucg = nc.dram_tensor("mucg", [1, MPC], bf16, kind="ExternalInput").ap()
    bq_d = nc.dram_tensor("bq", [MPC], f32, kind="ExternalInput").ap()
    bk_d = nc.dram_tensor("bk", [MPC], f32, kind="ExternalInput").ap()
    bv_d = nc.dram_tensor("bv", [MPC], f32, kind="ExternalInput").ap()
    bg_d = nc.dram_tensor("bgt", [MPC], f32, kind="ExternalInput").ap()
    ident = nc.dram_tensor("ident", [128, 128], bf16, kind="ExternalInput").ap()
    out_d = nc.dram_tensor("out", [MPC, LQ], f32, kind="ExternalOutput").ap()

    EPS = 1024.0 * 1024.0 * 1e-5

    with tile.TileContext(nc) as tc:
        import contextlib
        es = contextlib.ExitStack()
        with es:
            const = es.enter_context(tc.tile_pool(name="const", bufs=1))
            persist = es.enter_context(tc.tile_pool(name="persist", bufs=1))

            ones = const.tile([128, 128], bf16)
            nc.gpsimd.memset(ones[:, :], 1.0)
            eps_t = const.tile([128, 1], f32)
            nc.gpsimd.memset(eps_t[:, :], 1e-5)
            idt = const.tile([128, 128], bf16)
            nc.sync.dma_start(out=idt[:, :], in_=ident[:, :])

            # weights: [128, 8, 256] layouts (d-chunk, cols)
            wts = {}
            for nm, dr in (("q", wqT), ("k", wkT), ("v", wvT), ("g", wgT)):
                t = const.tile([128, N_DCH, MPC], bf16, tag=f"w{nm}")
                for j in range(N_DCH):
                    nc.sync.dma_start(out=t[:, j, :], in_=dr[128 * j:128 * (j + 1), :])
                wts[nm] = t
            wo_t = const.tile([128, 2, D], bf16)
            for mc in range(2):
                nc.sync.dma_start(out=wo_t[:, mc, :], in_=woT[128 * mc:128 * (mc + 1), :])
            mucs = {}
            for nm, dr in (("q", mucq), ("k", muck), ("v", mucv), ("g", mucg)):
                t = const.tile([1, MPC], bf16, tag=f"muc{nm}")
                nc.sync.dma_start(out=t[:, :], in_=dr[:, :])
                mucs[nm] = t
            biases = {}
            for nm, dr in (("q", bq_d), ("k", bk_d), ("v", bv_d), ("g", bg_d)):
                t = const.tile([128, 2], f32, tag=f"b{nm}")
                nc.sync.dma_start(out=t[:, :], in_=dr.rearrange("(c p) -> p c", p=128))
                biases[nm] = t

            # persistent activation outputs
            qhT = persist.tile([128, 2, LQ], bf16, tag="qhT")
            khT = persist.tile([128, 2, LQ], bf16, tag="khT")
            gT = persist.tile([128, 2, LQ], bf16, tag="gT")
            ygT = persist.tile([128, 2, LQ], bf16, tag="ygT")
            vaug = persist.tile([128, N_TCH, HPC, 65], bf16, tag="vaug")
            # contiguous memset; v-transposes later overwrite cols 0:64, col 64 stays 1.0
            nc.gpsimd.memset(vaug[:, :, :, :], 1.0)

            with tc.tile_pool(name="ph1", bufs=2) as ph1, \
                 tc.tile_pool(name="ph1b", bufs=1) as ph1b, \
                 tc.tile_pool(name="stage", bufs=1) as stage, \
                 tc.tile_pool(name="scr", bufs=2) as scr, \
                 tc.tile_pool(name="ps1", bufs=2, space="PSUM") as ps1, \
                 tc.tile_pool(name="pstr", bufs=2, space="PSUM") as pstr:

                def load_T(x_dram):
                    """load [2048,1024] f32 -> bf16 transposed xT [128,(i,j,t)]."""
                    xT = ph1.tile([128, N_TCH, N_DCH, 128], bf16, tag="xT")
                    xr = stage.tile([128, N_TCH, D], bf16, tag="xrow")
                    # 4 cast-DMAs (2.1MB each): out[p, i, c] = x[512q + 128i + p, c]
                    for qq in range(4):
                        nc.gpsimd.dma_start(
                            out=xr[:, 4 * qq:4 * (qq + 1), :],
                            in_=x_dram[512 * qq:512 * (qq + 1), :].rearrange(
                                "(i p) c -> p i c", p=128))
                        for i in range(4 * qq, 4 * qq + 4):
                            nc.sync.dma_start(out=xT[:, i, :, :], in_=xr[:, i, :], transpose=True)
                    return xT

                def rhs_slice(xT, j, tt):
                    return xT[:, 4 * tt:4 * (tt + 1), j, :]

                def stats(xT):
                    """returns r_rep [128,2048] f32 (1/sqrt(n*S2-S1^2+n^2 eps)), mu [1,2048] bf16"""
                    r_rep = ph1b.tile([128, LQ], f32, tag="r_rep")
                    mu = ph1b.tile([1, LQ], bf16, tag="mu")
                    for tt in range(N_TT):
                        sl = slice(512 * tt, 512 * (tt + 1))
                        s1 = ps1.tile([128, 512], f32, tag="s1")
                        s2 = ps1.tile([128, 512], f32, tag="s2")
                        for j in range(N_DCH):
                            nc.tensor.matmul(s1[:, :], ones[:, :], rhs_slice(xT, j, tt),
                                             start=(j == 0), stop=(j == N_DCH - 1))
                        for j in range(N_DCH):
                            sq = scr.tile([128, 512], bf16, tag="sq")
                            nc.vector.tensor_mul(sq[:, :], rhs_slice(xT, j, tt), rhs_slice(xT, j, tt))
                            nc.tensor.matmul(s2[:, :], ones[:, :], sq[:, :],
                                             start=(j == 0), stop=(j == N_DCH - 1))
                        s1s = scr.tile([128, 512], f32, tag="s1s")
                        nc.vector.tensor_copy(s1s[:, :], s1[:, :])
                        t1 = scr.tile([128, 512], f32, tag="t1")
                        nc.vector.tensor_mul(t1[:, :], s1s[:, :], s1s[:, :])
                        t2 = scr.tile([128, 512], f32, tag="t2")
                        nc.vector.scalar_tensor_tensor(t2[:, :], s2[:, :], 1024.0, t1[:, :], MUL, SUB)
                        t3 = scr.tile([128, 512], f32, tag="t3")
                        nc.scalar.activation(t3[:, :], t2[:, :], AF.Sqrt, bias=eps_t[:, :], scale=1.0 / (1024.0 * 1024.0))
                        nc.vector.reciprocal(r_rep[:, sl], t3[:, :])
                        nc.vector.tensor_scalar(mu[0:1, sl], s1s[0:1, :], 1.0 / 1024.0, None, op0=MUL)
                    return r_rep, mu

                def project(xT, wkey, muckey, mu, r_rep, out_t, sigmoid=False):
                    """out_t[:, mc, :] (bf16) = drain((x-mu)@W'^T * r) [*1024 + bias]"""
                    w = wts[wkey]
                    mc_t = mucs[muckey]
                    bias = biases[muckey if not sigmoid else "g"]
                    for mc in range(2):
                        for tt in range(N_TT):
                            sl = slice(512 * tt, 512 * (tt + 1))
                            pp = ps1.tile([128, 512], f32, tag="pp")
                            for j in range(N_DCH):
                                nc.tensor.matmul(pp[:, :], w[:, j, 128 * mc:128 * (mc + 1)],
                                                 rhs_slice(xT, j, tt), start=(j == 0), stop=False)
                            nc.tensor.matmul(pp[:, :], mc_t[:, 128 * mc:128 * (mc + 1)],
                                             mu[0:1, sl], start=False, stop=True)
                            if sigmoid:
                                tmp = scr.tile([128, 512], f32, tag="ptmp")
                                nc.vector.tensor_mul(tmp[:, :], pp[:, :], r_rep[:, sl])
                                nc.scalar.activation(out_t[:, mc, sl], tmp[:, :], AF.Sigmoid,
                                                     bias=bias[:, mc:mc + 1], scale=1.0)
                            else:
                                nc.vector.scalar_tensor_tensor(
                                    out_t[:, mc, sl], pp[:, :], bias[:, mc:mc + 1],
                                    r_rep[:, sl], ADD, MUL)

                # ---- q ----
                xT = load_T(xq)
                r_rep, mu = stats(xT)
                project(xT, "q", "q", mu, r_rep, qhT)
                project(xT, "g", "g", mu, r_rep, gT, sigmoid=True)
                # ---- k ----
                xT = load_T(xk)
                r_rep, mu = stats(xT)
                project(xT, "k", "k", mu, r_rep, khT)
                # ---- v ----
                xT = load_T(xv)
                r_rep, mu = stats(xT)
                vhT = ph1b.tile([128, 2, LQ], bf16, tag="vhT")
                project(xT, "v", "v", mu, r_rep, vhT)
                # transpose vhT -> vaug
                for mc in range(2):
                    for s in range(N_TCH):
                        pt = pstr.tile([128, 128], bf16, tag="ptr")
                        nc.tensor.transpose(pt[:, :], vhT[:, mc, 128 * s:128 * (s + 1)], idt[:, :])
                        nc.vector.tensor_copy(vaug[:, s, 2 * mc, 0:64], pt[:, 0:64])
                        nc.vector.tensor_copy(vaug[:, s, 2 * mc + 1, 0:64], pt[:, 64:128])

            # ---- attention: head pairs; 4 independent (head x tt) chains of
            # [128,512] ST -> exp -> O for fine-grained PE/ACT pipelining
            with tc.tile_pool(name="att", bufs=2) as att, \
                 tc.tile_pool(name="ps_st", bufs=1, space="PSUM") as ps_st, \
                 tc.tile_pool(name="ps_o", bufs=1, space="PSUM") as ps_o:
                for hp in range(2):
                    kc = hp
                    for th in range(2):
                        t0 = 1024 * th
                        o_ps = {}
                        for hb in range(2):
                            for tt in range(2):
                                o_ps[hb, tt] = ps_o.tile([65, 512], f32, name=f"o{hb}{tt}", tag=f"o{hb}{tt}")
                        for s in range(N_TCH):
                            for hb in range(2):
                                r0 = 64 * hb
                                for tt in range(2):
                                    sl_t = slice(t0 + 512 * tt, t0 + 512 * (tt + 1))
                                    st = ps_st.tile([128, 512], f32, tag=f"st{hb}{tt}")
                                    nc.tensor.matmul(st[:, :],
                                                     khT[r0:r0 + 64, kc, 128 * s:128 * (s + 1)],
                                                     qhT[r0:r0 + 64, kc, sl_t],
                                                     start=True, stop=True)
                                    pt = att.tile([128, 512], bf16, tag=f"pt{hb}{tt}")
                                    nc.scalar.activation(pt[:, :], st[:, :], AF.Exp, scale=0.125)
                                    nc.tensor.matmul(o_ps[hb, tt][:, :],
                                                     vaug[:, s, 2 * hp + hb, :], pt[:, :],
                                                     start=(s == 0), stop=(s == N_TCH - 1))
                        for hb in range(2):
                            r0 = 64 * hb
                            for tt in range(2):
                                o_p = o_ps[hb, tt]
                                sl_y = slice(t0 + 512 * tt, t0 + 512 * (tt + 1))
                                li_f = att.tile([65, 512], f32, tag="lif")
                                nc.vector.reciprocal(li_f[64:65, :], o_p[64:65, :])
                                li_b = att.tile([65, 512], bf16, tag="lib")
                                nc.vector.tensor_copy(li_b[64:65, :], li_f[64:65, :])
                                bc = ps_st.tile([64, 512], f32, tag=f"st{hb}{tt}")
                                nc.tensor.matmul(bc[:, :], ones[64:65, 0:64],
                                                 li_b[64:65, :], start=True, stop=True)
                                bcs = att.tile([64, 512], f32, tag="bcs")
                                nc.vector.tensor_copy(bcs[:, :], bc[:, :])
                                tmp2 = att.tile([128, 512], f32, tag="tmp2")
                                nc.vector.tensor_mul(tmp2[0:64, :], o_p[0:64, :], bcs[:, :])
                                if r0 != 0:
                                    # partition shift 0->64 via SBUF->SBUF DMA
                                    nc.gpsimd.dma_start(out=tmp2[64:128, :], in_=tmp2[0:64, :])
                                nc.vector.tensor_mul(ygT[r0:r0 + 64, kc, sl_y],
                                                     tmp2[r0:r0 + 64, :],
                                                     gT[r0:r0 + 64, kc, sl_y])

            # ---- out-proj + reduce-scatter ----
            with tc.tile_pool(name="po", bufs=2, space="PSUM") as po_p, \
                 tc.tile_pool(name="od", bufs=4) as od_p, \
                 tc.tile_pool(name="dram", bufs=1, space="DRAM") as dram_p:
                outb = dram_p.tile([D, LQ], f32, tag="outb")
                outrs = dram_p.tile([MPC, LQ], f32, tag="outrs")
                for half in range(4):
                    for nk in range(2 * half, 2 * half + 2):
                        for tt in range(N_TT):
                            po = po_p.tile([128, 512], f32, tag="po")
                            for mc in range(2):
                                nc.tensor.matmul(po[:, :], wo_t[:, mc, 128 * nk:128 * (nk + 1)],
                                                 ygT[:, mc, 512 * tt:512 * (tt + 1)],
                                                 start=(mc == 0), stop=(mc == 1))
                            ot = od_p.tile([128, 512], f32, tag="ot")
                            nc.vector.tensor_copy(ot[:, :], po[:, :])
                            nc.sync.dma_start(
                                out=outb[128 * nk:128 * (nk + 1), 512 * tt:512 * (tt + 1)],
                                in_=ot[:, :])
                    nc.gpsimd.collective_compute(
                        "ReduceScatter", ADD,
                        replica_groups=[[0, 1, 2, 3], [4, 5, 6, 7]],
                        ins=[outb[256 * half:256 * (half + 1), :].opt()],
                        outs=[outrs[64 * half:64 * (half + 1), :].opt()],
                    )
                for ch in range(2):
                    ob = od_p.tile([128, LQ], f32, tag="ob")
                    nc.sync.dma_start(out=ob[:, :], in_=outrs[128 * ch:128 * (ch + 1), :])
                    nc.sync.dma_start(out=out_d[128 * ch:128 * (ch + 1), :], in_=ob[:, :])

    nc.compile()
    return nc


def kernel(q, k, v, qln_g, qln_b, kvln_g, kvln_b, Wq, Wk, Wv, Wg, bg, Wo):
    import concourse.mybir as mybir
    from concourse import bass_utils

    bf16 = mybir.dt.np(mybir.dt.bfloat16)
    q = np.asarray(q, np.float32)
    k = np.asarray(k, np.float32)
    v = np.asarray(v, np.float32)
    qln_g = np.asarray(qln_g, np.float32)
    qln_b = np.asarray(qln_b, np.float32)
    kvln_g = np.asarray(kvln_g, np.float32)
    kvln_b = np.asarray(kvln_b, np.float32)
    Wq, Wk, Wv = np.asarray(Wq, np.float32), np.asarray(Wk, np.float32), np.asarray(Wv, np.float32)
    Wg, Wo = np.asarray(Wg, np.float32), np.asarray(Wo, np.float32)
    bg = np.asarray(bg, np.float32)

    # fold LN gamma into weights; beta into bias vectors
    Wqp, Wgp = Wq * qln_g[None, :], Wg * qln_g[None, :]
    Wkp, Wvp = Wk * kvln_g[None, :], Wv * kvln_g[None, :]
    bq_f, bk_f, bv_f = Wq @ qln_b, Wk @ kvln_b, Wv @ kvln_b
    bg_f = Wg @ qln_b + bg
    idm = np.eye(128, dtype=np.float32)

    if _NC_CACHE[0] is None:
        _NC_CACHE[0] = _build()
    nc = _NC_CACHE[0]

    in_maps = []
    for c in range(NC):
        beta, g = c // GPC, c % GPC
        m0 = MPC * g
        sl = slice(m0, m0 + MPC)
        in_maps.append({
            "xq": q[beta], "xk": k[beta], "xv": v[beta],
            "wqT": Wqp[sl, :].T.astype(bf16), "wkT": Wkp[sl, :].T.astype(bf16),
            "wvT": Wvp[sl, :].T.astype(bf16), "wgT": Wgp[sl, :].T.astype(bf16),
            "woT": Wo[:, sl].T.astype(bf16),
            "mucq": -Wqp[sl, :].sum(1)[None, :].astype(bf16),
            "muck": -Wkp[sl, :].sum(1)[None, :].astype(bf16),
            "mucv": -Wvp[sl, :].sum(1)[None, :].astype(bf16),
            "mucg": -Wgp[sl, :].sum(1)[None, :].astype(bf16),
            "bq": bq_f[sl], "bk": bk_f[sl], "bv": bv_f[sl], "bgt": bg_f[sl],
            "ident": idm.astype(bf16),
        })
    global _last_in_maps
    _last_in_maps = in_maps
    res = bass_utils.run_bass_kernel_spmd(nc, in_maps, core_ids=list(range(NC)))
    out = np.empty((B, LQ, D), np.float32)
    for beta in range(B):
        rows = np.empty((D, LQ), np.float32)
        for qtr in range(4):
            for g in range(GPC):
                rows[256 * qtr + 64 * g:256 * qtr + 64 * (g + 1)] = \
                    res.results[GPC * beta + g]["out"][64 * qtr:64 * (qtr + 1)]
        out[beta] = rows.T
    return out



# revision 9
# speedup vs baseline: 1.0896x; 1.0896x over previous
"""MHA Bass kernel for TRN2, 8 NeuronCores — v2.

Sharding: data-parallel on batch (2) x tensor-parallel on heads (4 groups of 4
heads). Core c handles batch c//4 and heads 4*(c%4)..4*(c%4)+3.

v2 design vs v1 baseline (770us):
- Host pre-casts q/k/v to bf16 (halves HBM load bytes; no cast-DMA).
- LayerNorm stats via DVE bn_stats/bn_aggr on token-major tiles + ACT rsqrt +
  one tensor_scalar normalize (removes all stats matmuls and the rank-1
  mu/r_rep machinery from PE; projections become plain GEMMs + bias).
- Normalized x transposed via xbar DMA-transpose, then projected.
- Attention: S^T scores per (head-half, key-chunk) into [128,1024] PSUM, one
  wide exp per tile on ACT, PV with ones-augmented V (denominator rides M=65).
- Softmax denominator reciprocal via DVE reciprocal_approx_fast (no ACT table
  switching), broadcast via a 1-row matmul into a reused st-PSUM region.
- Per-(hp,hb) yg tiles at partitions 0:64 (no partition-shift DMAs; gate rows
  for hb=1 pre-shifted once via sb2sb DMA).
- Out-projection after 4 small bf16 AllGathers of yg (one per (th,hp) quarter,
  issued as soon as each quarter of yg is ready) instead of 4 serial f32
  ReduceScatters at the end; oproj of th=0 is slotted into the middle of
  th=1's attention.
"""
import numpy as np

B, LQ, D = 2, 2048, 1024
NHEAD, DHEAD = 16, 64
NC = 8
GPC = 4              # cores per batch group
MPC = 256            # output cols per core
N_DCH = D // 128     # 8 d-chunks
N_TCH = LQ // 128    # 16 token chunks
HPC = 4              # heads per core

_NC_CACHE = [None]


def _build():
    import concourse.bacc as bacc
    import concourse.mybir as mybir
    from concourse import tile

    f32, bf16 = mybir.dt.float32, mybir.dt.bfloat16
    AF = mybir.ActivationFunctionType
    MUL, ADD, SUB = mybir.AluOpType.mult, mybir.AluOpType.add, mybir.AluOpType.subtract
    BYP = mybir.AluOpType.bypass

    nc = bacc.Bacc("TRN2", target_bir_lowering=False, debug=False, num_devices=NC)

    xq = nc.dram_tensor("xq", [LQ, D], bf16, kind="ExternalInput").ap()
    xk = nc.dram_tensor("xk", [LQ, D], bf16, kind="ExternalInput").ap()
    xv = nc.dram_tensor("xv", [LQ, D], bf16, kind="ExternalInput").ap()
    wqT = nc.dram_tensor("wqT", [D, MPC], bf16, kind="ExternalInput").ap()
    wkT = nc.dram_tensor("wkT", [D, MPC], bf16, kind="ExternalInput").ap()
    wvT = nc.dram_tensor("wvT", [D, MPC], bf16, kind="ExternalInput").ap()
    wgT = nc.dram_tensor("wgT", [D, MPC], bf16, kind="ExternalInput").ap()
    woA = nc.dram_tensor("woA", [D, MPC], bf16, kind="ExternalInput").ap()
    bq_d = nc.dram_tensor("bq", [MPC], f32, kind="ExternalInput").ap()
    bk_d = nc.dram_tensor("bk", [MPC], f32, kind="ExternalInput").ap()
    bv_d = nc.dram_tensor("bv", [MPC], f32, kind="ExternalInput").ap()
    bg_d = nc.dram_tensor("bgt", [MPC], f32, kind="ExternalInput").ap()
    ident = nc.dram_tensor("ident", [128, 128], bf16, kind="ExternalInput").ap()
    out_d = nc.dram_tensor("out", [MPC, LQ], f32, kind="ExternalOutput").ap()

    RG = [[0, 1, 2, 3], [4, 5, 6, 7]]

    with tile.TileContext(nc) as tc:
        import contextlib
        es = contextlib.ExitStack()
        with es:
            const = es.enter_context(tc.tile_pool(name="const", bufs=1))
            persist = es.enter_context(tc.tile_pool(name="persist", bufs=1))
            dram_p = es.enter_context(tc.tile_pool(name="dram", bufs=1, space="DRAM"))

            eps_t = const.tile([128, 1], f32)
            nc.gpsimd.memset(eps_t[:, :], 1e-5)
            ones64 = const.tile([65, 64], bf16)
            nc.gpsimd.memset(ones64[:, :], 1.0)
            idt = const.tile([128, 128], bf16)
            nc.sync.dma_start(out=idt[:, :], in_=ident[:, :])

            # weights [128, 8, 256]: chunk j holds W'[d_out, 128j+p] for in-dim
            wts = {}
            for nm, dr in (("q", wqT), ("k", wkT), ("v", wvT), ("g", wgT), ("o", woA)):
                t = const.tile([128, N_DCH, MPC], bf16, tag=f"w{nm}")
                for j in range(N_DCH):
                    nc.sync.dma_start(out=t[:, j, :], in_=dr[128 * j:128 * (j + 1), :])
                wts[nm] = t
            biases = {}
            for nm, dr in (("q", bq_d), ("k", bk_d), ("v", bv_d), ("g", bg_d)):
                t = const.tile([128, 2], f32, tag=f"b{nm}")
                nc.sync.dma_start(out=t[:, :], in_=dr.rearrange("(c p) -> p c", p=128))
                biases[nm] = t

            # persistent activations
            qhT = persist.tile([128, 2, LQ], bf16, tag="qhT")
            khT = persist.tile([128, 2, LQ], bf16, tag="khT")
            gT = persist.tile([128, 2, LQ], bf16, tag="gT")
            gR = persist.tile([64, 2, LQ], bf16, tag="gR")      # gate rows 64:128 shifted to 0:64
            vaug = persist.tile([128, N_TCH, HPC, 65], bf16, tag="vaug")
            nc.gpsimd.memset(vaug[:, :, :, :], 1.0)
            yg_sb = persist.tile([64, 4, 1024], bf16, tag="yg_sb")  # slot 2*hp+hb

            # DRAM staging for the collectives
            ygd = {}
            ygg = {}
            for th in range(2):
                for hp in range(2):
                    ygd[th, hp] = dram_p.tile([128, 1024], bf16, tag=f"ygd{th}{hp}",
                                              name=f"ygd{th}{hp}")
                    ygg[th, hp] = dram_p.tile([512, 1024], bf16, tag=f"ygg{th}{hp}",
                                              name=f"ygg{th}{hp}")

            # ---------------- phase 1: LN + projections ----------------
            with tc.tile_pool(name="ph1", bufs=1) as ph1, \
                 tc.tile_pool(name="ps1", bufs=1, space="PSUM") as ps1:

                vhT = ph1.tile([128, 2, LQ], bf16, tag="vhT")

                def do_input(x_dram, projs):
                    """load token-major, LN-normalize, transpose, project.
                    projs: list of (wkey, dst, sigmoid)"""
                    for qq in range(4):
                        xin = ph1.tile([128, 4, D], bf16, tag="xin", bufs=3)
                        nc.gpsimd.dma_start(
                            out=xin[:, :, :],
                            in_=x_dram[512 * qq:512 * (qq + 1), :].rearrange(
                                "(i p) c -> p i c", p=128))
                        st6 = ph1.tile([128, 4, 12], f32, tag="st6", bufs=2)
                        ag2 = ph1.tile([128, 4, 2], f32, tag="ag2", bufs=2)
                        rr = ph1.tile([128, 4], f32, tag="rr", bufs=2)
                        for il in range(4):
                            nc.vector.bn_stats(st6[:, il, 0:6], xin[:, il, 0:512])
                            nc.vector.bn_stats(st6[:, il, 6:12], xin[:, il, 512:1024])
                            nc.vector.bn_aggr(ag2[:, il, :], st6[:, il, :])
                        sq = ph1.tile([128, 4], f32, tag="sq", bufs=2)
                        nc.scalar.activation(sq[:, :], ag2[:, :, 1], AF.Sqrt,
                                             bias=eps_t[:, :], scale=1.0)
                        nc.vector.reciprocal_approx_fast(rr[:, :], sq[:, :])
                        xn = ph1.tile([128, 4, D], bf16, tag="xn", bufs=2)
                        xT = ph1.tile([128, 4, N_DCH, 128], bf16, tag="xT", bufs=3)
                        for il in range(4):
                            nc.vector.tensor_scalar(
                                xn[:, il, :], xin[:, il, :],
                                ag2[:, il, 0:1], rr[:, il:il + 1], op0=SUB, op1=MUL)
                            nc.sync.dma_start(out=xT[:, il, :, :], in_=xn[:, il, :],
                                              transpose=True)
                        # projections for this quarter (512 tokens)
                        sl = slice(512 * qq, 512 * (qq + 1))
                        for wkey, dst, sigmoid in projs:
                            w = wts[wkey]
                            for mc in range(2):
                                pp = ps1.tile([128, 512], f32, tag="pp", bufs=4)
                                for j in range(N_DCH):
                                    nc.tensor.matmul(pp[:, :],
                                                     w[:, j, 128 * mc:128 * (mc + 1)],
                                                     xT[:, :, j, :],
                                                     start=(j == 0), stop=(j == N_DCH - 1))
                                if sigmoid:
                                    nc.scalar.activation(dst[:, mc, sl], pp[:, :],
                                                         AF.Sigmoid,
                                                         bias=biases["g"][:, mc:mc + 1],
                                                         scale=1.0)
                                else:
                                    nc.vector.tensor_scalar(
                                        dst[:, mc, sl], pp[:, :],
                                        biases[wkey][:, mc:mc + 1], None, op0=ADD)
                        if projs[0][0] == "v":
                            # transpose v quarter into vaug as soon as projected
                            for mc in range(2):
                                for s in range(4 * qq, 4 * qq + 4):
                                    ptr = ps1.tile([128, 128], bf16, tag="ptr", bufs=2)
                                    nc.tensor.transpose(
                                        ptr[:, :], vhT[:, mc, 128 * s:128 * (s + 1)],
                                        idt[:, :])
                                    nc.vector.tensor_copy(vaug[:, s, 2 * mc, 0:64],
                                                          ptr[:, 0:64])
                                    nc.vector.tensor_copy(vaug[:, s, 2 * mc + 1, 0:64],
                                                          ptr[:, 64:128])

                do_input(xq, [("q", qhT, False), ("g", gT, True)])
                do_input(xk, [("k", khT, False)])
                do_input(xv, [("v", vhT, False)])
                # gate rows for hb=1, shifted down to partitions 0:64
                for hp in range(2):
                    nc.gpsimd.dma_start(out=gR[:, hp, :], in_=gT[64:128, hp, :])

            # ---------------- attention + out-proj ----------------
            with tc.tile_pool(name="att", bufs=1) as att, \
                 tc.tile_pool(name="ps_st", bufs=1, space="PSUM") as ps_st, \
                 tc.tile_pool(name="ps_o", bufs=1, space="PSUM") as ps_o, \
                 tc.tile_pool(name="ps_pp", bufs=1, space="PSUM") as ps_pp:

                def attention_block(th, hp):
                    q0 = 1024 * th
                    o_ps = {}
                    for hb in range(2):
                        for tt in range(2):
                            o_ps[hb, tt] = ps_o.tile([128, 512], f32, tag=f"o{hb}{tt}",
                                                     name=f"o{hb}{tt}")
                    for s in range(N_TCH):
                        for hb in range(2):
                            r0 = 64 * hb
                            st = ps_st.tile([128, 1024], f32, tag="st")
                            for tt in range(2):
                                nc.tensor.matmul(
                                    st[:, 512 * tt:512 * (tt + 1)],
                                    khT[r0:r0 + 64, hp, 128 * s:128 * (s + 1)],
                                    qhT[r0:r0 + 64, hp, q0 + 512 * tt:q0 + 512 * (tt + 1)],
                                    start=True, stop=True)
                            pt = att.tile([128, 1024], bf16, tag="pt", bufs=2)
                            nc.scalar.activation(pt[:, :], st[:, :], AF.Exp, scale=0.125)
                            for tt in range(2):
                                nc.tensor.matmul(
                                    o_ps[hb, tt][0:65, :],
                                    vaug[:, s, 2 * hp + hb, :],
                                    pt[:, 512 * tt:512 * (tt + 1)],
                                    start=(s == 0), stop=(s == N_TCH - 1))
                    # epilogue: normalize + gate into yg_sb
                    stv = ps_st.tile([128, 1024], f32, tag="st")
                    for hb in range(2):
                        for tt in range(2):
                            u = o_ps[hb, tt]
                            li = att.tile([65, 512], bf16, tag="li", bufs=2)
                            nc.vector.tensor_copy(li[64:65, :], u[64:65, :])
                            bc = stv[0:64, 512 * tt:512 * (tt + 1)]
                            nc.tensor.matmul(bc, ones64[64:65, 0:64], li[64:65, :],
                                             start=True, stop=True)
                            bcs = att.tile([64, 512], f32, tag="bcs", bufs=2)
                            nc.vector.reciprocal_approx_fast(bcs[:, :], bc)
                            tmp = att.tile([64, 512], bf16, tag="tmp", bufs=2)
                            nc.vector.tensor_mul(tmp[:, :], u[0:64, :], bcs[:, :])
                            qsl = slice(q0 + 512 * tt, q0 + 512 * (tt + 1))
                            gate = gT[0:64, hp, qsl] if hb == 0 else gR[:, hp, qsl]
                            nc.vector.tensor_mul(
                                yg_sb[:, 2 * hp + hb, 512 * tt:512 * (tt + 1)],
                                tmp[:, :], gate)
                    # ship this (th, hp) quarter of yg and AllGather it
                    for hb in range(2):
                        nc.sync.dma_start(out=ygd[th, hp][64 * hb:64 * (hb + 1), :],
                                          in_=yg_sb[:, 2 * hp + hb, :])
                    nc.gpsimd.collective_compute(
                        "AllGather", BYP, replica_groups=RG,
                        ins=[ygd[th, hp][:, :].opt()],
                        outs=[ygg[th, hp][:, :].opt()],
                    )

                def oproj_block(th):
                    q0 = 1024 * th
                    yggs = att.tile([128, 2, 4, 1024], bf16, tag="yggs", bufs=2)
                    for hp in range(2):
                        nc.gpsimd.dma_start(
                            out=yggs[:, hp, :, :],
                            in_=ygg[th, hp][:, :].rearrange("(g p) c -> p g c", p=128))
                    outs = att.tile([128, 2, 1024], f32, tag="outs", bufs=2)
                    for mc in range(2):
                        for tt in range(2):
                            pp = ps_pp.tile([128, 512], f32, tag="pp2", bufs=2)
                            for j in range(N_DCH):
                                nc.tensor.matmul(
                                    pp[:, :],
                                    wts["o"][:, j, 128 * mc:128 * (mc + 1)],
                                    yggs[:, j % 2, j // 2, 512 * tt:512 * (tt + 1)],
                                    start=(j == 0), stop=(j == N_DCH - 1))
                            nc.vector.tensor_copy(
                                outs[:, mc, 512 * tt:512 * (tt + 1)], pp[:, :])
                    for mc in range(2):
                        nc.sync.dma_start(
                            out=out_d[128 * mc:128 * (mc + 1), q0:q0 + 1024],
                            in_=outs[:, mc, :])

                attention_block(0, 0)
                attention_block(0, 1)
                attention_block(1, 0)
                oproj_block(0)   # AG(0,*) long done; PE slots this mid-attention
                attention_block(1, 1)
                oproj_block(1)

    nc.compile()
    return nc


def kernel(q, k, v, qln_g, qln_b, kvln_g, kvln_b, Wq, Wk, Wv, Wg, bg, Wo):
    import concourse.mybir as mybir
    from concourse import bass_utils

    bf16 = mybir.dt.np(mybir.dt.bfloat16)
    q = np.asarray(q, np.float32)
    k = np.asarray(k, np.float32)
    v = np.asarray(v, np.float32)
    qln_g = np.asarray(qln_g, np.float32)
    qln_b = np.asarray(qln_b, np.float32)
    kvln_g = np.asarray(kvln_g, np.float32)
    kvln_b = np.asarray(kvln_b, np.float32)
    Wq, Wk, Wv = np.asarray(Wq, np.float32), np.asarray(Wk, np.float32), np.asarray(Wv, np.float32)
    Wg, Wo = np.asarray(Wg, np.float32), np.asarray(Wo, np.float32)
    bg = np.asarray(bg, np.float32)

    # fold LN gamma into weights; beta into bias vectors
    Wqp, Wgp = Wq * qln_g[None, :], Wg * qln_g[None, :]
    Wkp, Wvp = Wk * kvln_g[None, :], Wv * kvln_g[None, :]
    bq_f, bk_f, bv_f = Wq @ qln_b, Wk @ kvln_b, Wv @ kvln_b
    bg_f = Wg @ qln_b + bg
    idm = np.eye(128, dtype=np.float32)

    if _NC_CACHE[0] is None:
        _NC_CACHE[0] = _build()
    nc = _NC_CACHE[0]

    xb = [x.astype(bf16) for x in (q, k, v)]
    in_maps = []
    for c in range(NC):
        beta, g = c // GPC, c % GPC
        m0 = MPC * g
        sl = slice(m0, m0 + MPC)
        in_maps.append({
            "xq": xb[0][beta], "xk": xb[1][beta], "xv": xb[2][beta],
            "wqT": Wqp[sl, :].T.astype(bf16), "wkT": Wkp[sl, :].T.astype(bf16),
            "wvT": Wvp[sl, :].T.astype(bf16), "wgT": Wgp[sl, :].T.astype(bf16),
            "woA": Wo[sl, :].T.astype(bf16),
            "bq": bq_f[sl], "bk": bk_f[sl], "bv": bv_f[sl], "bgt": bg_f[sl],
            "ident": idm.astype(bf16),
        })
    global _last_in_maps
    _last_in_maps = in_maps
    res = bass_utils.run_bass_kernel_spmd(nc, in_maps, core_ids=list(range(NC)))
    out = np.empty((B, LQ, D), np.float32)
    for beta in range(B):
        for g in range(GPC):
            out[beta][:, MPC * g:MPC * (g + 1)] = res.results[GPC * beta + g]["out"].T
    return out


# revision 12
# speedup vs baseline: 1.2491x; 1.1464x over previous
"""MHA Bass kernel for TRN2, 8 NeuronCores — v2.

Sharding: data-parallel on batch (2) x tensor-parallel on heads (4 groups of 4
heads). Core c handles batch c//4 and heads 4*(c%4)..4*(c%4)+3.

v2 design vs v1 baseline (770us):
- Host pre-casts q/k/v to bf16 (halves HBM load bytes; no cast-DMA).
- LayerNorm stats via DVE bn_stats/bn_aggr on token-major tiles + ACT rsqrt +
  one tensor_scalar normalize (removes all stats matmuls and the rank-1
  mu/r_rep machinery from PE; projections become plain GEMMs + bias).
- Normalized x transposed via xbar DMA-transpose, then projected.
- Attention: S^T scores per (head-half, key-chunk) into [128,1024] PSUM, one
  wide exp per tile on ACT, PV with ones-augmented V (denominator rides M=65).
- Softmax denominator reciprocal via DVE reciprocal_approx_fast (no ACT table
  switching), broadcast via a 1-row matmul into a reused st-PSUM region.
- Per-(hp,hb) yg tiles at partitions 0:64 (no partition-shift DMAs; gate rows
  for hb=1 pre-shifted once via sb2sb DMA).
- Out-projection after 4 small bf16 AllGathers of yg (one per (th,hp) quarter,
  issued as soon as each quarter of yg is ready) instead of 4 serial f32
  ReduceScatters at the end; oproj of th=0 is slotted into the middle of
  th=1's attention.
"""
import numpy as np

B, LQ, D = 2, 2048, 1024
NHEAD, DHEAD = 16, 64
NC = 8
GPC = 4              # cores per batch group
MPC = 256            # output cols per core
N_DCH = D // 128     # 8 d-chunks
N_TCH = LQ // 128    # 16 token chunks
HPC = 4              # heads per core

_NC_CACHE = [None]


def _build():
    import concourse.bacc as bacc
    import concourse.mybir as mybir
    from concourse import tile

    f32, bf16 = mybir.dt.float32, mybir.dt.bfloat16
    AF = mybir.ActivationFunctionType
    MUL, ADD, SUB = mybir.AluOpType.mult, mybir.AluOpType.add, mybir.AluOpType.subtract
    BYP = mybir.AluOpType.bypass

    nc = bacc.Bacc("TRN2", target_bir_lowering=False, debug=False, num_devices=NC)

    xq = nc.dram_tensor("xq", [LQ, D], bf16, kind="ExternalInput").ap()
    xk = nc.dram_tensor("xk", [LQ, D], bf16, kind="ExternalInput").ap()
    xv = nc.dram_tensor("xv", [LQ, D], bf16, kind="ExternalInput").ap()
    wqT = nc.dram_tensor("wqT", [D, MPC], bf16, kind="ExternalInput").ap()
    wkT = nc.dram_tensor("wkT", [D, MPC], bf16, kind="ExternalInput").ap()
    wvT = nc.dram_tensor("wvT", [D, MPC], bf16, kind="ExternalInput").ap()
    wgT = nc.dram_tensor("wgT", [D, MPC], bf16, kind="ExternalInput").ap()
    woA = nc.dram_tensor("woA", [D, MPC], bf16, kind="ExternalInput").ap()
    bq_d = nc.dram_tensor("bq", [MPC], f32, kind="ExternalInput").ap()
    bk_d = nc.dram_tensor("bk", [MPC], f32, kind="ExternalInput").ap()
    bv_d = nc.dram_tensor("bv", [MPC], f32, kind="ExternalInput").ap()
    bg_d = nc.dram_tensor("bgt", [MPC], f32, kind="ExternalInput").ap()
    ident = nc.dram_tensor("ident", [128, 128], bf16, kind="ExternalInput").ap()
    out_d = nc.dram_tensor("out", [MPC, LQ], f32, kind="ExternalOutput").ap()

    RG = [[0, 1, 2, 3], [4, 5, 6, 7]]

    with tile.TileContext(nc) as tc:
        import contextlib
        es = contextlib.ExitStack()
        with es:
            const = es.enter_context(tc.tile_pool(name="const", bufs=1))
            persist = es.enter_context(tc.tile_pool(name="persist", bufs=1))
            dram_p = es.enter_context(tc.tile_pool(name="dram", bufs=1, space="DRAM"))

            eps_t = const.tile([128, 1], f32)
            nc.gpsimd.memset(eps_t[:, :], 1e-5)
            ones64 = const.tile([65, 64], bf16)
            nc.gpsimd.memset(ones64[:, :], 1.0)
            idt = const.tile([128, 128], bf16)
            nc.sync.dma_start(out=idt[:, :], in_=ident[:, :])

            # weights [128, 8, 256]: chunk j holds W'[d_out, 128j+p] for in-dim
            wts = {}
            for nm, dr in (("q", wqT), ("k", wkT), ("v", wvT), ("g", wgT), ("o", woA)):
                t = const.tile([128, N_DCH, MPC], bf16, tag=f"w{nm}")
                for j in range(N_DCH):
                    nc.sync.dma_start(out=t[:, j, :], in_=dr[128 * j:128 * (j + 1), :])
                wts[nm] = t
            biases = {}
            for nm, dr in (("q", bq_d), ("k", bk_d), ("v", bv_d), ("g", bg_d)):
                t = const.tile([128, 2], f32, tag=f"b{nm}")
                nc.sync.dma_start(out=t[:, :], in_=dr.rearrange("(c p) -> p c", p=128))
                biases[nm] = t

            # persistent activations
            qhT = persist.tile([128, 2, LQ], bf16, tag="qhT")
            khT = persist.tile([128, 2, LQ], bf16, tag="khT")
            gT = persist.tile([128, 2, LQ], bf16, tag="gT")
            gR = persist.tile([64, 2, LQ], bf16, tag="gR")      # gate rows 64:128 shifted to 0:64
            vaug = persist.tile([128, N_TCH, HPC, 65], bf16, tag="vaug")
            nc.gpsimd.memset(vaug[:, :, :, :], 1.0)
            yg_sb = persist.tile([64, 4, 1024], bf16, tag="yg_sb")  # slot 2*hp+hb

            # DRAM staging for the collectives
            ygd = {}
            ygg = {}
            for th in range(2):
                for hp in range(2):
                    ygd[th, hp] = dram_p.tile([128, 1024], bf16, tag=f"ygd{th}{hp}",
                                              name=f"ygd{th}{hp}")
                    ygg[th, hp] = dram_p.tile([512, 1024], bf16, tag=f"ygg{th}{hp}",
                                              name=f"ygg{th}{hp}")

            # ---------------- phase 1: LN + projections ----------------
            with tc.tile_pool(name="ph1", bufs=1) as ph1, \
                 tc.tile_pool(name="ps1", bufs=1, space="PSUM") as ps1:

                vhT = ph1.tile([128, 2, LQ], bf16, tag="vhT")

                # flat (input, quarter) pipeline; projections lag the
                # LN/transpose chain by one step so PE never starves.
                steps = []
                for x_dram, projs in ((xq, [("q", qhT, False), ("g", gT, True)]),
                                      (xk, [("k", khT, False)]),
                                      (xv, [("v", vhT, False)])):
                    for qq in range(4):
                        steps.append((x_dram, projs, qq))

                def emit_chain(x_dram, qq):
                    xin = ph1.tile([128, 4, D], bf16, tag="xin", bufs=4, name="xin")
                    nc.gpsimd.dma_start(
                        out=xin[:, :, :],
                        in_=x_dram[512 * qq:512 * (qq + 1), :].rearrange(
                            "(i p) c -> p i c", p=128))
                    st6 = ph1.tile([128, 4, 12], f32, tag="st6", bufs=3, name="st6")
                    ag2 = ph1.tile([128, 4, 2], f32, tag="ag2", bufs=3, name="ag2")
                    rr = ph1.tile([128, 4], f32, tag="rr", bufs=3, name="rr")
                    for il in range(4):
                        nc.vector.bn_stats(st6[:, il, 0:6], xin[:, il, 0:512])
                        nc.vector.bn_stats(st6[:, il, 6:12], xin[:, il, 512:1024])
                        nc.vector.bn_aggr(ag2[:, il, :], st6[:, il, :])
                    sq = ph1.tile([128, 4], f32, tag="sq", bufs=3, name="sq")
                    nc.scalar.activation(sq[:, :], ag2[:, :, 1], AF.Sqrt,
                                         bias=eps_t[:, :], scale=1.0)
                    nc.vector.reciprocal_approx_fast(rr[:, :], sq[:, :])
                    xn = ph1.tile([128, 4, D], bf16, tag="xn", bufs=3, name="xn")
                    xT = ph1.tile([128, 4, N_DCH, 128], bf16, tag="xT", bufs=4,
                                  name="xT")
                    for il in range(4):
                        nc.vector.tensor_scalar(
                            xn[:, il, :], xin[:, il, :],
                            ag2[:, il, 0:1], rr[:, il:il + 1], op0=SUB, op1=MUL)
                        nc.sync.dma_start(out=xT[:, il, :, :], in_=xn[:, il, :],
                                          transpose=True)
                    return xT

                def emit_proj(projs, qq, xT):
                    sl = slice(512 * qq, 512 * (qq + 1))
                    for wkey, dst, sigmoid in projs:
                        w = wts[wkey]
                        for mc in range(2):
                            pp = ps1.tile([128, 512], f32, tag="pp", bufs=4, name="pp")
                            for j in range(N_DCH):
                                nc.tensor.matmul(pp[:, :],
                                                 w[:, j, 128 * mc:128 * (mc + 1)],
                                                 xT[:, :, j, :],
                                                 start=(j == 0), stop=(j == N_DCH - 1))
                            if sigmoid:
                                nc.scalar.activation(dst[:, mc, sl], pp[:, :],
                                                     AF.Sigmoid,
                                                     bias=biases["g"][:, mc:mc + 1],
                                                     scale=1.0)
                            else:
                                nc.vector.tensor_scalar(
                                    dst[:, mc, sl], pp[:, :],
                                    biases[wkey][:, mc:mc + 1], None, op0=ADD)
                    if projs[0][0] == "v":
                        for mc in range(2):
                            for s in range(4 * qq, 4 * qq + 4):
                                ptr = ps1.tile([128, 128], bf16, tag="ptr", bufs=2,
                                               name="ptr")
                                nc.tensor.transpose(
                                    ptr[:, :], vhT[:, mc, 128 * s:128 * (s + 1)],
                                    idt[:, :])
                                nc.vector.tensor_copy(vaug[:, s, 2 * mc, 0:64],
                                                      ptr[:, 0:64])
                                nc.vector.tensor_copy(vaug[:, s, 2 * mc + 1, 0:64],
                                                      ptr[:, 64:128])

                pending = None
                for x_dram, projs, qq in steps:
                    xT = emit_chain(x_dram, qq)
                    if pending is not None:
                        emit_proj(*pending)
                    pending = (projs, qq, xT)
                emit_proj(*pending)
                # gate rows for hb=1, shifted down to partitions 0:64
                for hp in range(2):
                    nc.gpsimd.dma_start(out=gR[:, hp, :], in_=gT[64:128, hp, :])

            # ---------------- attention + out-proj ----------------
            with tc.tile_pool(name="att", bufs=1) as att, \
                 tc.tile_pool(name="ps_st", bufs=1, space="PSUM") as ps_st, \
                 tc.tile_pool(name="ps_o", bufs=1, space="PSUM") as ps_o:

                def attention_block(th, hp):
                    q0 = 1024 * th
                    o_ps = {}
                    for hb in range(2):
                        for tt in range(2):
                            o_ps[hb, tt] = ps_o.tile([128, 512], f32, tag=f"o{hb}{tt}",
                                                     name=f"o{hb}{tt}")
                    for s in range(N_TCH):
                        for hb in range(2):
                            r0 = 64 * hb
                            st = ps_st.tile([128, 1024], f32, tag=f"st{hb}",
                                            name=f"st{hb}")
                            for tt in range(2):
                                nc.tensor.matmul(
                                    st[:, 512 * tt:512 * (tt + 1)],
                                    khT[r0:r0 + 64, hp, 128 * s:128 * (s + 1)],
                                    qhT[r0:r0 + 64, hp, q0 + 512 * tt:q0 + 512 * (tt + 1)],
                                    start=True, stop=True)
                            pt = att.tile([128, 1024], bf16, tag=f"pt{hb}", bufs=2,
                                          name=f"pt{hb}")
                            nc.scalar.activation(pt[:, :], st[:, :], AF.Exp, scale=0.125)
                            for tt in range(2):
                                nc.tensor.matmul(
                                    o_ps[hb, tt][0:65, :],
                                    vaug[:, s, 2 * hp + hb, :],
                                    pt[:, 512 * tt:512 * (tt + 1)],
                                    start=(s == 0), stop=(s == N_TCH - 1))
                    # epilogue: normalize + gate into yg_sb
                    stv = ps_st.tile([128, 1024], f32, tag="st0", name="stv")
                    for hb in range(2):
                        for tt in range(2):
                            u = o_ps[hb, tt]
                            li = att.tile([65, 512], bf16, tag="li", bufs=2)
                            nc.vector.tensor_copy(li[64:65, :], u[64:65, :])
                            bc = stv[0:64, 512 * tt:512 * (tt + 1)]
                            nc.tensor.matmul(bc, ones64[64:65, 0:64], li[64:65, :],
                                             start=True, stop=True)
                            bcs = att.tile([64, 512], f32, tag="bcs", bufs=2)
                            nc.vector.reciprocal_approx_fast(bcs[:, :], bc)
                            tmp = att.tile([64, 512], bf16, tag="tmp", bufs=2)
                            nc.vector.tensor_mul(tmp[:, :], u[0:64, :], bcs[:, :])
                            qsl = slice(q0 + 512 * tt, q0 + 512 * (tt + 1))
                            gate = gT[0:64, hp, qsl] if hb == 0 else gR[:, hp, qsl]
                            nc.vector.tensor_mul(
                                yg_sb[:, 2 * hp + hb, 512 * tt:512 * (tt + 1)],
                                tmp[:, :], gate)
                    # ship this (th, hp) quarter of yg and AllGather it
                    for hb in range(2):
                        nc.sync.dma_start(out=ygd[th, hp][64 * hb:64 * (hb + 1), :],
                                          in_=yg_sb[:, 2 * hp + hb, :])
                    nc.gpsimd.collective_compute(
                        "AllGather", BYP, replica_groups=RG,
                        ins=[ygd[th, hp][:, :].opt()],
                        outs=[ygg[th, hp][:, :].opt()],
                    )

                def oproj_block(th):
                    q0 = 1024 * th
                    yggs = att.tile([128, 2, 4, 1024], bf16, tag="yggs", bufs=2)
                    for hp in range(2):
                        nc.gpsimd.dma_start(
                            out=yggs[:, hp, :, :],
                            in_=ygg[th, hp][:, :].rearrange("(g p) c -> p g c", p=128))
                    outs = att.tile([128, 2, 1024], f32, tag="outs", bufs=2)
                    for mc in range(2):
                        for tt in range(2):
                            pp = ps_o.tile([128, 512], f32, tag=f"o{mc}{tt}",
                                           name="pp2")
                            for j in range(N_DCH):
                                nc.tensor.matmul(
                                    pp[:, :],
                                    wts["o"][:, j, 128 * mc:128 * (mc + 1)],
                                    yggs[:, j % 2, j // 2, 512 * tt:512 * (tt + 1)],
                                    start=(j == 0), stop=(j == N_DCH - 1))
                            nc.vector.tensor_copy(
                                outs[:, mc, 512 * tt:512 * (tt + 1)], pp[:, :])
                    for mc in range(2):
                        nc.sync.dma_start(
                            out=out_d[128 * mc:128 * (mc + 1), q0:q0 + 1024],
                            in_=outs[:, mc, :])

                attention_block(0, 0)
                attention_block(0, 1)
                attention_block(1, 0)
                oproj_block(0)   # AG(0,*) long done; PE slots this mid-attention
                attention_block(1, 1)
                oproj_block(1)

    nc.compile()
    return nc


def kernel(q, k, v, qln_g, qln_b, kvln_g, kvln_b, Wq, Wk, Wv, Wg, bg, Wo):
    import concourse.mybir as mybir
    from concourse import bass_utils

    bf16 = mybir.dt.np(mybir.dt.bfloat16)
    q = np.asarray(q, np.float32)
    k = np.asarray(k, np.float32)
    v = np.asarray(v, np.float32)
    qln_g = np.asarray(qln_g, np.float32)
    qln_b = np.asarray(qln_b, np.float32)
    kvln_g = np.asarray(kvln_g, np.float32)
    kvln_b = np.asarray(kvln_b, np.float32)
    Wq, Wk, Wv = np.asarray(Wq, np.float32), np.asarray(Wk, np.float32), np.asarray(Wv, np.float32)
    Wg, Wo = np.asarray(Wg, np.float32), np.asarray(Wo, np.float32)
    bg = np.asarray(bg, np.float32)

    # fold LN gamma into weights; beta into bias vectors
    Wqp, Wgp = Wq * qln_g[None, :], Wg * qln_g[None, :]
    Wkp, Wvp = Wk * kvln_g[None, :], Wv * kvln_g[None, :]
    bq_f, bk_f, bv_f = Wq @ qln_b, Wk @ kvln_b, Wv @ kvln_b
    bg_f = Wg @ qln_b + bg
    idm = np.eye(128, dtype=np.float32)

    if _NC_CACHE[0] is None:
        _NC_CACHE[0] = _build()
    nc = _NC_CACHE[0]

    xb = [x.astype(bf16) for x in (q, k, v)]
    in_maps = []
    for c in range(NC):
        beta, g = c // GPC, c % GPC
        m0 = MPC * g
        sl = slice(m0, m0 + MPC)
        in_maps.append({
            "xq": xb[0][beta], "xk": xb[1][beta], "xv": xb[2][beta],
            "wqT": Wqp[sl, :].T.astype(bf16), "wkT": Wkp[sl, :].T.astype(bf16),
            "wvT": Wvp[sl, :].T.astype(bf16), "wgT": Wgp[sl, :].T.astype(bf16),
            "woA": Wo[sl, :].T.astype(bf16),
            "bq": bq_f[sl], "bk": bk_f[sl], "bv": bv_f[sl], "bgt": bg_f[sl],
            "ident": idm.astype(bf16),
        })
    global _last_in_maps
    _last_in_maps = in_maps
    res = bass_utils.run_bass_kernel_spmd(nc, in_maps, core_ids=list(range(NC)))
    out = np.empty((B, LQ, D), np.float32)
    for beta in range(B):
        for g in range(GPC):
            out[beta][:, MPC * g:MPC * (g + 1)] = res.results[GPC * beta + g]["out"].T
    return out
